# revision 3
# baseline (speedup 1.0000x reference)
"""AdaptiveEmbedding on 8 TRN2 NeuronCores.

Data-parallel over tokens; tables + projections replicated in device DRAM as
bf16.  sqrt(1024)=32 is folded into table0 and the projections on the host,
so the device never scales explicitly.

Measured cost structure (HW traces):
  - per-gathered-row cost is dominated by SWDGE descriptor GENERATION on the
    Q7 (~9 ns/row, serial on the Pool engine), not the transfer;
    single_packet=True coalesces each gather's descriptors into one packet
    per DMA engine and makes the drain ~9x faster (safe while
    num_idxs/16 <= 64 descriptors per engine packet).
  - ~7us framework preamble + ~10.7us one-time gather-ucode IRAM load gate
    the first desc-gen.
  - PE runs [128]x[128x512] bf16 matmuls at ~380-600ns each (mid p-state).

Design:
  - tokens of every bucket are dealt ROUND-ROBIN over the 8 cores per
    (bucket, 32768-row chunk) unit, so per-core unit counts differ by at
    most 1 and the SPMD caps (max over cores, padded to 128) are minimal;
    the host unshard places rows by global token position.
  - bucket 0 (d=1024, prescaled on host): plain SWDGE gather of 2KB rows,
    desc-gen issued first, rows DMA'd straight back out (no compute).
  - buckets 1-3: transposed SWDGE gathers (single_packet, queue 1) land
    lhsT directly as [d, tokens]; per 128-token group: 2 matmuls (N=512,
    K=d) against the replicated projection, PSUM->SBUF bf16 copy split
    512/512 across DVE and ACT, per-unit contiguous output writes; the
    smallest unit is processed last to minimize the tail.

Output is bf16 on device (rel err ~3e-3), upcast to f32 on host.
"""
import math
import numpy as np
import ml_dtypes

N_VOCAB = 267735
STARTS = [0, 20000, 40000, 200000]
ENDS = [20000, 40000, 200000, N_VOCAB]
N_EMBEDS = [1024, 256, 64, 16]
N_CORES = 8
NEMB = 1024
SCALE = 32.0  # sqrt(1024), folded into table0/proj on host
CHUNK = 32768  # int16-addressable rows per SWDGE gather chunk
P = 128
MM_N = 512  # matmul rhs free size (<= one PSUM bank = 512 f32)

# SWDGE units: (bucket, chunk) for buckets 1..3
UNITS = []
for _b in (1, 2, 3):
    _nr = ENDS[_b] - STARTS[_b]
    for _c in range(math.ceil(_nr / CHUNK)):
        UNITS.append((_b, _c))


def _wrap16(a):
    # [N] -> [16, N/16] wrapped, replicated to 128 partitions
    w = a.reshape(-1, 16).T.astype(np.int16)
    return np.tile(w, (8, 1))


def _prep_host(inputs):
    x = np.asarray(inputs["x"]).astype(np.int64).reshape(-1)
    ntok = x.size
    assert ntok % N_CORES == 0
    per = ntok // N_CORES

    bf = ml_dtypes.bfloat16
    # bucket 0: prescaled, native 1024-wide rows (indirect gather)
    tab0 = np.ascontiguousarray(
        (np.asarray(inputs["table0"], np.float32) * SCALE).astype(bf)
    )
    # buckets 1-3: pad rows to 128 elements (256B) for SWDGE, split chunks
    tabs = {}
    for b in (1, 2, 3):
        t = np.asarray(inputs[f"table{b}"], np.float32)
        d = N_EMBEDS[b]
        if d < P:
            tp = np.zeros((t.shape[0], P), np.float32)
            tp[:, :d] = t
            t = tp
        tb = t.astype(bf)
        nr = t.shape[0]
        for c in range(math.ceil(nr / CHUNK)):
            tabs[(b, c)] = np.ascontiguousarray(tb[c * CHUNK : (c + 1) * CHUNK])
    projs = {
        i: np.ascontiguousarray(
            (np.asarray(inputs[f"proj{i}"], np.float32) * SCALE).astype(bf)
        )
        for i in (1, 2, 3)
    }

    # globally balanced dealing: each unit's tokens (over the FULL x) are
    # dealt round-robin to the 8 cores, so per-core unit counts differ by at
    # most 1 and caps are minimal.  pos entries are GLOBAL token indices; the
    # host unshard places rows by global position.
    lists = [{} for _ in range(N_CORES)]  # core -> {unit or 0: (lid, pos)}
    caps = {}

    def deal(key, gpos, glid):
        mx = 0
        for core in range(N_CORES):
            lid = glid[core::N_CORES]
            pos = gpos[core::N_CORES]
            lists[core][key] = (lid, pos)
            mx = max(mx, len(pos))
        return max(P, -(-mx // P) * P)

    gpos0 = np.nonzero(x < ENDS[0])[0]
    cap0 = deal(0, gpos0, x[gpos0])
    for (b, c) in UNITS:
        lo = STARTS[b] + c * CHUNK
        hi = min(STARTS[b] + (c + 1) * CHUNK, ENDS[b])
        gpos = np.nonzero((x >= lo) & (x < hi))[0]
        caps[(b, c)] = deal((b, c), gpos, x[gpos] - lo)
    G0 = cap0 // P

    # int16 SWDGE meta: per unit, wrapped idx block
    m16off = {}
    off = 0
    for u in UNITS:
        m16off[u] = off
        off += caps[u] // 16
    m16w = off
    metas16 = []
    metas32 = []
    for core in range(N_CORES):
        cols = []
        for u in UNITS:
            lid, _ = lists[core][u]
            il = np.zeros(caps[u], np.int64)
            il[: len(lid)] = lid
            cols.append(_wrap16(il))
        metas16.append(np.concatenate(cols, axis=1))
        lid0, _ = lists[core][0]
        i0 = np.zeros(cap0, np.int64)
        i0[: len(lid0)] = lid0
        metas32.append(_wrap16(i0))
    return per, tab0, tabs, projs, metas16, metas32, caps, cap0, m16off, m16w, lists


def _build(per, tabs_shapes, projs, caps, cap0, m16off, m16w):
    import concourse.bass as bass
    import concourse.tile as tile
    from concourse import bacc, mybir

    bf = mybir.dt.bfloat16
    f32 = mybir.dt.float32
    nc = bacc.Bacc("TRN2", target_bir_lowering=False, debug=False,
                   num_swdge_queues=2)

    G0 = cap0 // P
    tab0_d = nc.dram_tensor("tab0", list(tabs_shapes[0]), bf, kind="ExternalInput")
    tab_d = {
        u: nc.dram_tensor(f"tab{u[0]}_{u[1]}", list(tabs_shapes[u]), bf,
                          kind="ExternalInput")
        for u in UNITS
    }
    proj_d = {
        i: nc.dram_tensor(f"proj{i}", list(projs[i].shape), bf,
                          kind="ExternalInput")
        for i in (1, 2, 3)
    }
    meta16_d = nc.dram_tensor("meta16", [P, m16w], mybir.dt.int16,
                              kind="ExternalInput")
    meta32_d = nc.dram_tensor("meta32", [P, cap0 // 16], mybir.dt.int16,
                              kind="ExternalInput")
    ncap = cap0 + sum(caps.values())
    outc = nc.dram_tensor("outc", [ncap, NEMB], bf, kind="ExternalOutput")
    coff = {0: 0}
    off = cap0
    for u in UNITS:
        coff[u] = off
        off += caps[u]

    with tile.TileContext(nc) as tc:
        with (
            tc.tile_pool(name="sb", bufs=1) as sb,
            tc.tile_pool(name="pso", bufs=4, space="PSUM") as pso,
        ):
            # warmup: tiny transposed gather with no data deps -- prefetches
            # the SWDGE gather ucode IRAM load under the meta/proj DMAs
            widx = sb.tile([P, 8], mybir.dt.int16, tag="widx")
            nc.gpsimd.memset(widx[:], 0)
            wout = sb.tile([P, 2 * P], bf, tag="wout")
            nc.gpsimd.dma_gather(
                out_ap=wout[:].rearrange("p (k n) -> p k n", n=P),
                in_ap=tab_d[(1, 0)].ap(),
                idxs_ap=widx[:],
                num_idxs=P,
                num_idxs_reg=P,
                elem_size=256,
                transpose=True,
                single_packet=True,
                queue_num=1,
            )

            meta16_t = sb.tile([P, m16w], mybir.dt.int16, tag="meta16")
            nc.sync.dma_start(meta16_t[:], meta16_d.ap())
            meta32_t = sb.tile([P, cap0 // 16], mybir.dt.int16, tag="meta32")
            nc.sync.dma_start(meta32_t[:], meta32_d.ap())

            # projection tiles (K on partitions)
            p1a = sb.tile([P, NEMB], bf, tag="p1a")
            nc.scalar.dma_start(p1a[:], proj_d[1].ap()[0:128, :])
            p1b = sb.tile([P, NEMB], bf, tag="p1b")
            nc.scalar.dma_start(p1b[:], proj_d[1].ap()[128:256, :])
            p2 = sb.tile([64, NEMB], bf, tag="p2")
            nc.scalar.dma_start(p2[:], proj_d[2].ap())
            p3 = sb.tile([16, NEMB], bf, tag="p3")
            nc.scalar.dma_start(p3[:], proj_d[3].ap())
            prt = {1: (p1a, p1b), 2: (p2,), 3: (p3,)}

            # SWDGE transposed gathers: et[(u)] = [128(k), KC, cap] lhsT tiles
            et = {}
            unit_order = [u for u in UNITS if u[0] == 2] + \
                         [u for u in UNITS if u[0] == 1] + \
                         [u for u in UNITS if u[0] == 3]

            def emit_gather(u):
                b, c = u
                cap = caps[u]
                d = N_EMBEDS[b]
                dp = max(d, P)
                KC = dp // P
                t = sb.tile([P, KC * cap], bf, tag=f"g{u[0]}_{u[1]}",
                            name=f"g{u[0]}_{u[1]}")
                et[u] = t
                nc.gpsimd.dma_gather(
                    out_ap=t[:].rearrange("p (k n) -> p k n", n=cap),
                    in_ap=tab_d[u].ap(),
                    idxs_ap=meta16_t[:, m16off[u] : m16off[u] + cap // 16],
                    num_idxs=cap,
                    num_idxs_reg=cap,
                    elem_size=dp,
                    transpose=True,
                    single_packet=True,
                    queue_num=1,
                )

            # b0 first: plain gather of prescaled 2KB rows, straight out
            # (drain overlaps the b1-3 desc-gens; write streams early)
            g0 = sb.tile([P, G0 * NEMB], bf, tag="g0")
            nc.gpsimd.dma_gather(
                out_ap=g0[:].rearrange("p (g e) -> p g e", e=NEMB),
                in_ap=tab0_d.ap(),
                idxs_ap=meta32_t[:, 0 : cap0 // 16],
                num_idxs=cap0,
                num_idxs_reg=cap0,
                elem_size=NEMB,
                single_packet=False,
                queue_num=1,
            )
            nc.sync.dma_start(
                outc.ap()[0:cap0, :].rearrange("(g p) e -> p g e", p=P),
                g0[:].rearrange("p (g e) -> p g e", e=NEMB),
            )

            # b2 chunks next (feeds PE earliest)
            for u in unit_order:
                if u[0] == 2:
                    emit_gather(u)

            for u in unit_order:
                if u[0] != 2:
                    emit_gather(u)

            # per-group projection + copy + per-unit writes
            alt = 0
            for u in unit_order:
                b, c = u
                cap = caps[u]
                d = N_EMBEDS[b]
                dp = max(d, P)
                KC = dp // P
                dk = min(d, P)
                G = cap // P
                o = sb.tile([P, G * NEMB], bf, tag=f"o{u[0]}_{u[1]}",
                            name=f"o{u[0]}_{u[1]}")
                tv = et[u][:].rearrange("p (k n) -> p k n", n=cap)
                for g in range(G):
                    po = pso.tile([P, NEMB], f32, tag="po")
                    for n in range(NEMB // MM_N):
                        for kc in range(KC):
                            nc.tensor.matmul(
                                out=po[:, n * MM_N : (n + 1) * MM_N],
                                lhsT=tv[0:dk, kc, g * P : (g + 1) * P],
                                rhs=prt[b][kc][0:dk, n * MM_N : (n + 1) * MM_N],
                                start=(kc == 0),
                                stop=(kc == KC - 1),
                            )
                    dst = o[:, g * NEMB : (g + 1) * NEMB]
                    if alt % 2 == 0:
                        nc.vector.tensor_copy(out=dst[:, 0:512], in_=po[:, 0:512])
                        nc.scalar.copy(out=dst[:, 512:1024], in_=po[:, 512:1024])
                    else:
                        nc.scalar.copy(out=dst[:, 0:512], in_=po[:, 0:512])
                        nc.vector.tensor_copy(out=dst[:, 512:1024], in_=po[:, 512:1024])
                    alt += 1
                nc.sync.dma_start(
                    outc.ap()[coff[u] : coff[u] + cap, :].rearrange(
                        "(g p) e -> p g e", p=P),
                    o[:].rearrange("p (g e) -> p g e", e=NEMB),
                )
    nc.compile()
    return nc


def _ensure_profile_hook():
    try:
        import antenv.axon_hooks  # noqa: F401
        return
    except ImportError:
        pass
    import contextlib, ctypes, sys, types

    so_path = "/opt/axon/libaxon_pjrt.so"
    hook = None
    try:
        lib = ctypes.CDLL(so_path)
        if hasattr(lib, "axon_start_nrt_profile"):
            lib.axon_start_nrt_profile.argtypes = [
                ctypes.POINTER(ctypes.c_int64), ctypes.c_size_t]
            lib.axon_start_nrt_profile.restype = ctypes.c_int64
            lib.axon_stop_nrt_profile.argtypes = [ctypes.c_char_p]
            lib.axon_stop_nrt_profile.restype = ctypes.c_int64

            @contextlib.contextmanager
            def hook(output_dir, device_ids):
                import jax
                jax.devices()
                if device_ids:
                    ids = (ctypes.c_int64 * len(device_ids))(*device_ids)
                    rc = lib.axon_start_nrt_profile(ids, len(device_ids))
                else:
                    rc = lib.axon_start_nrt_profile(None, 0)
                if rc != 0:
                    raise RuntimeError(f"axon_start_nrt_profile rc={rc}")
                try:
                    yield
                finally:
                    lib.axon_stop_nrt_profile(str(output_dir).encode())
    except OSError:
        pass
    mod = types.ModuleType("antenv.axon_hooks")
    mod.get_axon_ntff_profile_hook = lambda: hook
    mod.set_axon_ntff_profile_hook = lambda h: None
    sys.modules["antenv.axon_hooks"] = mod


def _unshard(ntok, caps, cap0, lists, results):
    coff = {0: 0}
    off = cap0
    for u in UNITS:
        coff[u] = off
        off += caps[u]
    acc = np.zeros((ntok, NEMB), np.float32)
    for i in range(N_CORES):
        oc = np.asarray(results[i]["outc"])
        _, pos0 = lists[i][0]
        acc[pos0] = oc[0 : len(pos0)].astype(np.float32)
        for u in UNITS:
            _, pos = lists[i][u]
            acc[pos] = oc[coff[u] : coff[u] + len(pos)].astype(np.float32)
    return acc


def _run(inputs, trace=False):
    _ensure_profile_hook()
    from concourse.bass_utils import run_bass_kernel_spmd

    (per, tab0, tabs, projs, metas16, metas32, caps, cap0, m16off, m16w,
     lists) = _prep_host(inputs)
    shapes = {0: tab0.shape}
    shapes.update({u: tabs[u].shape for u in UNITS})
    nc = _build(per, shapes, projs, caps, cap0, m16off, m16w)

    in_maps = []
    for core in range(N_CORES):
        m = {f"tab{u[0]}_{u[1]}": np.asarray(tabs[u]) for u in UNITS}
        m["tab0"] = np.asarray(tab0)
        m.update({f"proj{i}": np.asarray(projs[i]) for i in (1, 2, 3)})
        m["meta16"] = metas16[core]
        m["meta32"] = metas32[core]
        in_maps.append(m)
    try:
        res = run_bass_kernel_spmd(
            nc, in_maps, core_ids=list(range(N_CORES)), trace=trace
        )
    except Exception:
        import time as _time

        _time.sleep(90)
        res = run_bass_kernel_spmd(
            nc, in_maps, core_ids=list(range(N_CORES)), trace=trace
        )
    x = np.asarray(inputs["x"])
    acc = _unshard(x.size, caps, cap0, lists, res.results)
    full = acc.reshape(*x.shape, NEMB)
    return full, res


def kernel(**inputs) -> np.ndarray:
    out, _ = _run(inputs, trace=False)
    return out


# revision 4
# speedup vs baseline: 1.0166x; 1.0166x over previous
"""AdaptiveEmbedding on 8 TRN2 NeuronCores — v3.

Data-parallel over tokens (4096/core); tables + projections replicated in
device DRAM as bf16.  sqrt(1024)=32 is folded into table0 and the
projections on the host, so the device never scales explicitly.

Bottleneck analysis of v1 (84.7us):
  - ~10us one-time IRAM ucode loads (2 libs: plain + transposed dma_gather)
  - ~21us serialized SWDGE descriptor generation on the Pool engine
  - PE busy ~58us: per-matmul ~600ns at the mid p-state (the PE clock ramps
    0.65 -> 1.2 -> 2.4 GHz with ~3us of continuous execution; gaps reset it)

v3 changes:
  - bucket 0 (d=1024, host-prescaled) is gathered with indirect_dma_start
    (HWDGE dynamic queue, one offset per partition, 128 rows/instr) and DMA'd
    straight back out -- no custom-op ucode, so only ONE IRAM lib remains
    (transposed dma_gather), prefetched by a tiny warmup gather issued first.
  - meta loads on the sync engine, not gpsimd.
  - buckets 1-3 keep v1's transposed SWDGE gathers (lhsT lands directly as
    [d, tokens]; no PE transposes), chunked by the int16 32768-row limit.
  - PSUM->SBUF copies are plain copies (scale prefolded), alternating
    DVE/ACT; matmul N=1024 per instruction (single po bank pair).
  - per-unit contiguous output writes (host places rows at token positions).
"""
import math
import numpy as np
import ml_dtypes

N_VOCAB = 267735
STARTS = [0, 20000, 40000, 200000]
ENDS = [20000, 40000, 200000, N_VOCAB]
N_EMBEDS = [1024, 256, 64, 16]
N_CORES = 8
NEMB = 1024
SCALE = 32.0  # sqrt(1024), folded into table0/proj on host
CHUNK = 32768  # int16-addressable rows per SWDGE gather chunk
P = 128
MM_N = 512  # matmul rhs free size (<= one PSUM bank = 512 f32)

# SWDGE units: (bucket, chunk) for buckets 1..3
UNITS = []
for _b in (1, 2, 3):
    _nr = ENDS[_b] - STARTS[_b]
    for _c in range(math.ceil(_nr / CHUNK)):
        UNITS.append((_b, _c))


def _wrap16(a):
    # [N] -> [16, N/16] wrapped, replicated to 128 partitions
    w = a.reshape(-1, 16).T.astype(np.int16)
    return np.tile(w, (8, 1))


def _prep_host(inputs):
    x = np.asarray(inputs["x"]).astype(np.int64).reshape(-1)
    ntok = x.size
    assert ntok % N_CORES == 0
    per = ntok // N_CORES

    bf = ml_dtypes.bfloat16
    # bucket 0: prescaled, native 1024-wide rows (indirect gather)
    tab0 = np.ascontiguousarray(
        (np.asarray(inputs["table0"], np.float32) * SCALE).astype(bf)
    )
    # buckets 1-3: pad rows to 128 elements (256B) for SWDGE, split chunks
    tabs = {}
    for b in (1, 2, 3):
        t = np.asarray(inputs[f"table{b}"], np.float32)
        d = N_EMBEDS[b]
        if d < P:
            tp = np.zeros((t.shape[0], P), np.float32)
            tp[:, :d] = t
            t = tp
        tb = t.astype(bf)
        nr = t.shape[0]
        for c in range(math.ceil(nr / CHUNK)):
            tabs[(b, c)] = np.ascontiguousarray(tb[c * CHUNK : (c + 1) * CHUNK])
    projs = {
        i: np.ascontiguousarray(
            (np.asarray(inputs[f"proj{i}"], np.float32) * SCALE).astype(bf)
        )
        for i in (1, 2, 3)
    }

    # globally balanced dealing: each unit's tokens (over the FULL x) are
    # dealt round-robin to the 8 cores, so per-core unit counts differ by at
    # most 1 and caps are minimal.  pos entries are GLOBAL token indices; the
    # host unshard places rows by global position.
    lists = [{} for _ in range(N_CORES)]  # core -> {unit or 0: (lid, pos)}
    caps = {}

    def deal(key, gpos, glid):
        mx = 0
        for core in range(N_CORES):
            lid = glid[core::N_CORES]
            pos = gpos[core::N_CORES]
            lists[core][key] = (lid, pos)
            mx = max(mx, len(pos))
        return max(P, -(-mx // P) * P)

    gpos0 = np.nonzero(x < ENDS[0])[0]
    cap0 = deal(0, gpos0, x[gpos0])
    n0 = max(16, -(-max(len(lists[c][0][1]) for c in range(N_CORES)) // 16) * 16)
    for (b, c) in UNITS:
        lo = STARTS[b] + c * CHUNK
        hi = min(STARTS[b] + (c + 1) * CHUNK, ENDS[b])
        gpos = np.nonzero((x >= lo) & (x < hi))[0]
        caps[(b, c)] = deal((b, c), gpos, x[gpos] - lo)
    G0 = cap0 // P

    # int16 SWDGE meta: per unit, wrapped idx block
    m16off = {}
    off = 0
    for u in UNITS:
        m16off[u] = off
        off += caps[u] // 16
    m16w = off
    metas16 = []
    metas32 = []
    for core in range(N_CORES):
        cols = []
        for u in UNITS:
            lid, _ = lists[core][u]
            il = np.zeros(caps[u], np.int64)
            il[: len(lid)] = lid
            cols.append(_wrap16(il))
        metas16.append(np.concatenate(cols, axis=1))
        lid0, _ = lists[core][0]
        i0 = np.zeros(cap0, np.int64)
        i0[: len(lid0)] = lid0
        metas32.append(_wrap16(i0))
    return (per, tab0, tabs, projs, metas16, metas32, caps, cap0, n0,
            m16off, m16w, lists)


def _build(per, tabs_shapes, projs, caps, cap0, n0, m16off, m16w):
    import concourse.bass as bass
    import concourse.tile as tile
    from concourse import bacc, mybir

    bf = mybir.dt.bfloat16
    f32 = mybir.dt.float32
    nc = bacc.Bacc("TRN2", target_bir_lowering=False, debug=False,
                   num_swdge_queues=2)

    G0 = cap0 // P
    tab0_d = nc.dram_tensor("tab0", list(tabs_shapes[0]), bf, kind="ExternalInput")
    tab_d = {
        u: nc.dram_tensor(f"tab{u[0]}_{u[1]}", list(tabs_shapes[u]), bf,
                          kind="ExternalInput")
        for u in UNITS
    }
    proj_d = {
        i: nc.dram_tensor(f"proj{i}", list(projs[i].shape), bf,
                          kind="ExternalInput")
        for i in (1, 2, 3)
    }
    meta16_d = nc.dram_tensor("meta16", [P, m16w], mybir.dt.int16,
                              kind="ExternalInput")
    meta32_d = nc.dram_tensor("meta32", [P, cap0 // 16], mybir.dt.int16,
                              kind="ExternalInput")
    ncap = cap0 + sum(caps.values())
    outc = nc.dram_tensor("outc", [ncap, NEMB], bf, kind="ExternalOutput")
    coff = {0: 0}
    off = cap0
    for u in UNITS:
        coff[u] = off
        off += caps[u]

    with tile.TileContext(nc) as tc:
        with (
            tc.tile_pool(name="sb", bufs=1) as sb,
            tc.tile_pool(name="pso", bufs=4, space="PSUM") as pso,
        ):
            # warmup: tiny transposed gather with no data deps -- prefetches
            # the SWDGE gather ucode IRAM load under the meta/proj DMAs
            widx = sb.tile([P, 1], mybir.dt.int16, tag="widx")
            nc.gpsimd.memset(widx[:], 0)
            wout = sb.tile([P, 256], bf, tag="wout")
            nc.gpsimd.dma_gather(
                out_ap=wout[:].rearrange("p (g e) -> p g e", e=256),
                in_ap=tab_d[(1, 0)].ap(),
                idxs_ap=widx[:],
                num_idxs=16,
                num_idxs_reg=16,
                elem_size=256,
                single_packet=True,
                queue_num=1,
            )

            meta16_t = sb.tile([P, m16w], mybir.dt.int16, tag="meta16")
            nc.sync.dma_start(meta16_t[:], meta16_d.ap())
            meta32_t = sb.tile([P, cap0 // 16], mybir.dt.int16, tag="meta32")
            nc.sync.dma_start(meta32_t[:], meta32_d.ap())

            # projection tiles (K on partitions)
            p1a = sb.tile([P, NEMB], bf, tag="p1a")
            nc.scalar.dma_start(p1a[:], proj_d[1].ap()[0:128, :])
            p1b = sb.tile([P, NEMB], bf, tag="p1b")
            nc.scalar.dma_start(p1b[:], proj_d[1].ap()[128:256, :])
            p2 = sb.tile([64, NEMB], bf, tag="p2")
            nc.scalar.dma_start(p2[:], proj_d[2].ap())
            p3 = sb.tile([16, NEMB], bf, tag="p3")
            nc.scalar.dma_start(p3[:], proj_d[3].ap())
            prt = {1: (p1a, p1b), 2: (p2,), 3: (p3,)}

            # SWDGE transposed gathers: et[(u)] = [128(k), KC, cap] lhsT tiles
            et = {}
            unit_order = [u for u in UNITS if u[0] == 2] + \
                         [u for u in UNITS if u[0] == 1] + \
                         [u for u in UNITS if u[0] == 3]

            def emit_gather(u):
                b, c = u
                cap = caps[u]
                d = N_EMBEDS[b]
                dp = max(d, P)
                KC = dp // P
                t = sb.tile([P, KC * cap], bf, tag=f"g{u[0]}_{u[1]}",
                            name=f"g{u[0]}_{u[1]}")
                et[u] = t
                nc.gpsimd.dma_gather(
                    out_ap=t[:].rearrange("p (k n) -> p k n", n=cap),
                    in_ap=tab_d[u].ap(),
                    idxs_ap=meta16_t[:, m16off[u] : m16off[u] + cap // 16],
                    num_idxs=cap,
                    num_idxs_reg=cap,
                    elem_size=dp,
                    transpose=True,
                    single_packet=True,
                    queue_num=1,
                )

            # b0 first: plain gather of prescaled 2KB rows, straight out
            # (drain overlaps the b1-3 desc-gens; write streams early)
            g0 = sb.tile([P, G0 * NEMB], bf, tag="g0")
            nc.gpsimd.dma_gather(
                out_ap=g0[:].rearrange("p (g e) -> p g e", e=NEMB),
                in_ap=tab0_d.ap(),
                idxs_ap=meta32_t[:, 0 : n0 // 16],
                num_idxs=n0,
                num_idxs_reg=n0,
                elem_size=NEMB,
                single_packet=False,
                queue_num=1,
            )
            nc.sync.dma_start(
                outc.ap()[0:cap0, :].rearrange("(g p) e -> p g e", p=P),
                g0[:].rearrange("p (g e) -> p g e", e=NEMB),
            )

            # b2 chunks next (feeds PE earliest)
            for u in unit_order:
                if u[0] == 2:
                    emit_gather(u)

            for u in unit_order:
                if u[0] != 2:
                    emit_gather(u)

            # per-group projection + copy + per-unit writes
            alt = 0
            for u in unit_order:
                b, c = u
                cap = caps[u]
                d = N_EMBEDS[b]
                dp = max(d, P)
                KC = dp // P
                dk = min(d, P)
                G = cap // P
                o = sb.tile([P, G * NEMB], bf, tag=f"o{u[0]}_{u[1]}",
                            name=f"o{u[0]}_{u[1]}")
                tv = et[u][:].rearrange("p (k n) -> p k n", n=cap)
                for g in range(G):
                    po = pso.tile([P, NEMB], f32, tag="po")
                    for n in range(NEMB // MM_N):
                        for kc in range(KC):
                            nc.tensor.matmul(
                                out=po[:, n * MM_N : (n + 1) * MM_N],
                                lhsT=tv[0:dk, kc, g * P : (g + 1) * P],
                                rhs=prt[b][kc][0:dk, n * MM_N : (n + 1) * MM_N],
                                start=(kc == 0),
                                stop=(kc == KC - 1),
                            )
                    dst = o[:, g * NEMB : (g + 1) * NEMB]
                    if alt % 2 == 0:
                        nc.vector.tensor_copy(out=dst[:, 0:512], in_=po[:, 0:512])
                        nc.scalar.copy(out=dst[:, 512:1024], in_=po[:, 512:1024])
                    else:
                        nc.scalar.copy(out=dst[:, 0:512], in_=po[:, 0:512])
                        nc.vector.tensor_copy(out=dst[:, 512:1024], in_=po[:, 512:1024])
                    alt += 1
                nc.sync.dma_start(
                    outc.ap()[coff[u] : coff[u] + cap, :].rearrange(
                        "(g p) e -> p g e", p=P),
                    o[:].rearrange("p (g e) -> p g e", e=NEMB),
                )
    nc.compile()
    return nc


def _ensure_profile_hook():
    try:
        import antenv.axon_hooks  # noqa: F401
        return
    except ImportError:
        pass
    import contextlib, ctypes, sys, types

    so_path = "/opt/axon/libaxon_pjrt.so"
    hook = None
    try:
        lib = ctypes.CDLL(so_path)
        if hasattr(lib, "axon_start_nrt_profile"):
            lib.axon_start_nrt_profile.argtypes = [
                ctypes.POINTER(ctypes.c_int64), ctypes.c_size_t]
            lib.axon_start_nrt_profile.restype = ctypes.c_int64
            lib.axon_stop_nrt_profile.argtypes = [ctypes.c_char_p]
            lib.axon_stop_nrt_profile.restype = ctypes.c_int64

            @contextlib.contextmanager
            def hook(output_dir, device_ids):
                import jax
                jax.devices()
                if device_ids:
                    ids = (ctypes.c_int64 * len(device_ids))(*device_ids)
                    rc = lib.axon_start_nrt_profile(ids, len(device_ids))
                else:
                    rc = lib.axon_start_nrt_profile(None, 0)
                if rc != 0:
                    raise RuntimeError(f"axon_start_nrt_profile rc={rc}")
                try:
                    yield
                finally:
                    lib.axon_stop_nrt_profile(str(output_dir).encode())
    except OSError:
        pass
    mod = types.ModuleType("antenv.axon_hooks")
    mod.get_axon_ntff_profile_hook = lambda: hook
    mod.set_axon_ntff_profile_hook = lambda h: None
    sys.modules["antenv.axon_hooks"] = mod


def _unshard(ntok, caps, cap0, lists, results):
    coff = {0: 0}
    off = cap0
    for u in UNITS:
        coff[u] = off
        off += caps[u]
    acc = np.zeros((ntok, NEMB), np.float32)
    for i in range(N_CORES):
        oc = np.asarray(results[i]["outc"])
        _, pos0 = lists[i][0]
        acc[pos0] = oc[0 : len(pos0)].astype(np.float32)
        for u in UNITS:
            _, pos = lists[i][u]
            acc[pos] = oc[coff[u] : coff[u] + len(pos)].astype(np.float32)
    return acc


def _run(inputs, trace=False):
    _ensure_profile_hook()
    from concourse.bass_utils import run_bass_kernel_spmd

    (per, tab0, tabs, projs, metas16, metas32, caps, cap0, n0, m16off,
     m16w, lists) = _prep_host(inputs)
    shapes = {0: tab0.shape}
    shapes.update({u: tabs[u].shape for u in UNITS})
    nc = _build(per, shapes, projs, caps, cap0, n0, m16off, m16w)

    in_maps = []
    for core in range(N_CORES):
        m = {f"tab{u[0]}_{u[1]}": np.asarray(tabs[u]) for u in UNITS}
        m["tab0"] = np.asarray(tab0)
        m.update({f"proj{i}": np.asarray(projs[i]) for i in (1, 2, 3)})
        m["meta16"] = metas16[core]
        m["meta32"] = metas32[core]
        in_maps.append(m)
    try:
        res = run_bass_kernel_spmd(
            nc, in_maps, core_ids=list(range(N_CORES)), trace=trace
        )
    except Exception:
        import time as _time

        _time.sleep(90)
        res = run_bass_kernel_spmd(
            nc, in_maps, core_ids=list(range(N_CORES)), trace=trace
        )
    x = np.asarray(inputs["x"])
    acc = _unshard(x.size, caps, cap0, lists, res.results)
    full = acc.reshape(*x.shape, NEMB)
    return full, res


def kernel(**inputs) -> np.ndarray:
    out, _ = _run(inputs, trace=False)
    return out


# revision 5
# speedup vs baseline: 1.0361x; 1.0192x over previous
"""AdaptiveEmbedding on 8 TRN2 NeuronCores.

Data-parallel over tokens; tables + projections replicated in device DRAM
as bf16.  sqrt(1024)=32 is folded into table0 and the projections on the
host, so the device never scales explicitly.

Measured cost structure (HW traces): the kernel is paced by SWDGE
descriptor GENERATION on the Q7 (~9 ns/gathered row, serial on the Pool
engine), behind a ~7us framework preamble + ~10.7us one-time gather-ucode
IRAM load.  single_packet=True coalesces each gather's descriptors into
one packet per DMA engine (~9x faster drain; safe while num_idxs/16 <= 64).

Design:
  - tokens of every bucket are dealt ROUND-ROBIN over the 8 cores per
    (bucket, 32768-row chunk) unit, so per-core counts differ by at most 1
    and SPMD caps are minimal; host unshard places rows by global position.
  - bucket 0 (d=1024, prescaled on host): plain SWDGE gather of 2KB rows
    (num_idxs trimmed to max-count rounded to 16 -- non-transposed gathers
    have no %128 constraint), desc-gen first, rows DMA'd straight back out.
  - buckets 1-3: transposed SWDGE gathers (single_packet, queue 1) land
    lhsT directly as [d, tokens]; per 128-token group 2 matmuls (N=512)
    against the replicated projection, PSUM->SBUF bf16 copies split
    512/512 across DVE and ACT, per-unit output writes, smallest unit last.

Output is bf16 on device (rel err ~3e-3), upcast to f32 on host.
"""
import math
import numpy as np
import ml_dtypes

N_VOCAB = 267735
STARTS = [0, 20000, 40000, 200000]
ENDS = [20000, 40000, 200000, N_VOCAB]
N_EMBEDS = [1024, 256, 64, 16]
N_CORES = 8
NEMB = 1024
SCALE = 32.0  # sqrt(1024), folded into table0/proj on host
CHUNK = 32768  # int16-addressable rows per SWDGE gather chunk
P = 128
MM_N = 512  # matmul rhs free size (<= one PSUM bank = 512 f32)

# SWDGE units: (bucket, chunk) for buckets 1..3
UNITS = []
for _b in (1, 2, 3):
    _nr = ENDS[_b] - STARTS[_b]
    for _c in range(math.ceil(_nr / CHUNK)):
        UNITS.append((_b, _c))


def _wrap16(a):
    # [N] -> [16, N/16] wrapped, replicated to 128 partitions
    w = a.reshape(-1, 16).T.astype(np.int16)
    return np.tile(w, (8, 1))


def _prep_host(inputs):
    x = np.asarray(inputs["x"]).astype(np.int64).reshape(-1)
    ntok = x.size
    assert ntok % N_CORES == 0
    per = ntok // N_CORES

    bf = ml_dtypes.bfloat16
    # bucket 0: prescaled, native 1024-wide rows (indirect gather)
    tab0 = np.ascontiguousarray(
        (np.asarray(inputs["table0"], np.float32) * SCALE).astype(bf)
    )
    # buckets 1-3: pad rows to 128 elements (256B) for SWDGE, split chunks
    tabs = {}
    for b in (1, 2, 3):
        t = np.asarray(inputs[f"table{b}"], np.float32)
        d = N_EMBEDS[b]
        if d < P:
            tp = np.zeros((t.shape[0], P), np.float32)
            tp[:, :d] = t
            t = tp
        tb = t.astype(bf)
        nr = t.shape[0]
        for c in range(math.ceil(nr / CHUNK)):
            tabs[(b, c)] = np.ascontiguousarray(tb[c * CHUNK : (c + 1) * CHUNK])
    projs = {
        i: np.ascontiguousarray(
            (np.asarray(inputs[f"proj{i}"], np.float32) * SCALE).astype(bf)
        )
        for i in (1, 2, 3)
    }

    # globally balanced dealing: each unit's tokens (over the FULL x) are
    # dealt round-robin to the 8 cores, so per-core unit counts differ by at
    # most 1 and caps are minimal.  pos entries are GLOBAL token indices; the
    # host unshard places rows by global position.
    lists = [{} for _ in range(N_CORES)]  # core -> {unit or 0: (lid, pos)}
    caps = {}

    def deal(key, gpos, glid):
        mx = 0
        for core in range(N_CORES):
            lid = glid[core::N_CORES]
            pos = gpos[core::N_CORES]
            lists[core][key] = (lid, pos)
            mx = max(mx, len(pos))
        return max(P, -(-mx // P) * P)

    gpos0 = np.nonzero(x < ENDS[0])[0]
    cap0 = deal(0, gpos0, x[gpos0])
    n0 = max(16, -(-max(len(lists[c][0][1]) for c in range(N_CORES)) // 16) * 16)
    for (b, c) in UNITS:
        lo = STARTS[b] + c * CHUNK
        hi = min(STARTS[b] + (c + 1) * CHUNK, ENDS[b])
        gpos = np.nonzero((x >= lo) & (x < hi))[0]
        caps[(b, c)] = deal((b, c), gpos, x[gpos] - lo)
    G0 = cap0 // P

    # int16 SWDGE meta: per unit, wrapped idx block
    m16off = {}
    off = 0
    for u in UNITS:
        m16off[u] = off
        off += caps[u] // 16
    m16w = off
    metas16 = []
    metas32 = []
    for core in range(N_CORES):
        cols = []
        for u in UNITS:
            lid, _ = lists[core][u]
            il = np.zeros(caps[u], np.int64)
            il[: len(lid)] = lid
            cols.append(_wrap16(il))
        metas16.append(np.concatenate(cols, axis=1))
        lid0, _ = lists[core][0]
        i0 = np.zeros(cap0, np.int64)
        i0[: len(lid0)] = lid0
        metas32.append(_wrap16(i0))
    return (per, tab0, tabs, projs, metas16, metas32, caps, cap0, n0,
            m16off, m16w, lists)


def _build(per, tabs_shapes, projs, caps, cap0, n0, m16off, m16w):
    import concourse.bass as bass
    import concourse.tile as tile
    from concourse import bacc, mybir

    bf = mybir.dt.bfloat16
    f32 = mybir.dt.float32
    nc = bacc.Bacc("TRN2", target_bir_lowering=False, debug=False,
                   num_swdge_queues=2)

    G0 = cap0 // P
    tab0_d = nc.dram_tensor("tab0", list(tabs_shapes[0]), bf, kind="ExternalInput")
    tab_d = {
        u: nc.dram_tensor(f"tab{u[0]}_{u[1]}", list(tabs_shapes[u]), bf,
                          kind="ExternalInput")
        for u in UNITS
    }
    proj_d = {
        i: nc.dram_tensor(f"proj{i}", list(projs[i].shape), bf,
                          kind="ExternalInput")
        for i in (1, 2, 3)
    }
    meta16_d = nc.dram_tensor("meta16", [P, m16w], mybir.dt.int16,
                              kind="ExternalInput")
    meta32_d = nc.dram_tensor("meta32", [P, cap0 // 16], mybir.dt.int16,
                              kind="ExternalInput")
    ncap = cap0 + sum(caps.values())
    outc = nc.dram_tensor("outc", [ncap, NEMB], bf, kind="ExternalOutput")
    coff = {0: 0}
    off = cap0
    for u in UNITS:
        coff[u] = off
        off += caps[u]

    with tile.TileContext(nc) as tc:
        with (
            tc.tile_pool(name="sb", bufs=1) as sb,
            tc.tile_pool(name="pso", bufs=4, space="PSUM") as pso,
        ):
            # warmup: tiny transposed gather with no data deps -- prefetches
            # the SWDGE gather ucode IRAM load under the meta/proj DMAs
            widx = sb.tile([P, 1], mybir.dt.int16, tag="widx")
            nc.gpsimd.memset(widx[:], 0)
            wout = sb.tile([P, 256], bf, tag="wout")
            nc.gpsimd.dma_gather(
                out_ap=wout[:].rearrange("p (g e) -> p g e", e=256),
                in_ap=tab_d[(1, 0)].ap(),
                idxs_ap=widx[:],
                num_idxs=16,
                num_idxs_reg=16,
                elem_size=256,
                single_packet=True,
                queue_num=1,
            )

            meta16_t = sb.tile([P, m16w], mybir.dt.int16, tag="meta16")
            nc.sync.dma_start(meta16_t[:], meta16_d.ap())
            meta32_t = sb.tile([P, cap0 // 16], mybir.dt.int16, tag="meta32")
            nc.sync.dma_start(meta32_t[:], meta32_d.ap())

            # projection tiles (K on partitions)
            p1a = sb.tile([P, NEMB], bf, tag="p1a")
            nc.scalar.dma_start(p1a[:], proj_d[1].ap()[0:128, :])
            p1b = sb.tile([P, NEMB], bf, tag="p1b")
            nc.scalar.dma_start(p1b[:], proj_d[1].ap()[128:256, :])
            p2 = sb.tile([64, NEMB], bf, tag="p2")
            nc.scalar.dma_start(p2[:], proj_d[2].ap())
            p3 = sb.tile([16, NEMB], bf, tag="p3")
            nc.scalar.dma_start(p3[:], proj_d[3].ap())
            prt = {1: (p1a, p1b), 2: (p2,), 3: (p3,)}

            # SWDGE transposed gathers: et[(u)] = [128(k), KC, cap] lhsT tiles
            et = {}
            unit_order = [u for u in UNITS if u[0] == 2] + \
                         [u for u in UNITS if u[0] == 1] + \
                         [u for u in UNITS if u[0] == 3]

            def emit_gather(u):
                b, c = u
                cap = caps[u]
                d = N_EMBEDS[b]
                dp = max(d, P)
                KC = dp // P
                t = sb.tile([P, KC * cap], bf, tag=f"g{u[0]}_{u[1]}",
                            name=f"g{u[0]}_{u[1]}")
                et[u] = t
                nc.gpsimd.dma_gather(
                    out_ap=t[:].rearrange("p (k n) -> p k n", n=cap),
                    in_ap=tab_d[u].ap(),
                    idxs_ap=meta16_t[:, m16off[u] : m16off[u] + cap // 16],
                    num_idxs=cap,
                    num_idxs_reg=cap,
                    elem_size=dp,
                    transpose=True,
                    single_packet=True,
                    queue_num=1,
                )

            # b0 first: plain gather of prescaled 2KB rows, straight out
            # (drain overlaps the b1-3 desc-gens; write streams early)
            g0 = sb.tile([P, G0 * NEMB], bf, tag="g0")
            nc.gpsimd.dma_gather(
                out_ap=g0[:].rearrange("p (g e) -> p g e", e=NEMB),
                in_ap=tab0_d.ap(),
                idxs_ap=meta32_t[:, 0 : n0 // 16],
                num_idxs=n0,
                num_idxs_reg=n0,
                elem_size=NEMB,
                single_packet=False,
                queue_num=1,
            )
            nc.sync.dma_start(
                outc.ap()[0:cap0, :].rearrange("(g p) e -> p g e", p=P),
                g0[:].rearrange("p (g e) -> p g e", e=NEMB),
            )

            # b2 chunks next (feeds PE earliest)
            for u in unit_order:
                if u[0] == 2:
                    emit_gather(u)

            for u in unit_order:
                if u[0] != 2:
                    emit_gather(u)

            # per-group projection + copy + per-unit writes
            alt = 0
            for u in unit_order:
                b, c = u
                cap = caps[u]
                d = N_EMBEDS[b]
                dp = max(d, P)
                KC = dp // P
                dk = min(d, P)
                G = cap // P
                o = sb.tile([P, G * NEMB], bf, tag=f"o{u[0]}_{u[1]}",
                            name=f"o{u[0]}_{u[1]}")
                tv = et[u][:].rearrange("p (k n) -> p k n", n=cap)
                for g in range(G):
                    po = pso.tile([P, NEMB], f32, tag="po")
                    for n in range(NEMB // MM_N):
                        for kc in range(KC):
                            nc.tensor.matmul(
                                out=po[:, n * MM_N : (n + 1) * MM_N],
                                lhsT=tv[0:dk, kc, g * P : (g + 1) * P],
                                rhs=prt[b][kc][0:dk, n * MM_N : (n + 1) * MM_N],
                                start=(kc == 0),
                                stop=(kc == KC - 1),
                            )
                    dst = o[:, g * NEMB : (g + 1) * NEMB]
                    if alt % 2 == 0:
                        nc.vector.tensor_copy(out=dst[:, 0:512], in_=po[:, 0:512])
                        nc.scalar.copy(out=dst[:, 512:1024], in_=po[:, 512:1024])
                    else:
                        nc.scalar.copy(out=dst[:, 0:512], in_=po[:, 0:512])
                        nc.vector.tensor_copy(out=dst[:, 512:1024], in_=po[:, 512:1024])
                    alt += 1
                nc.sync.dma_start(
                    outc.ap()[coff[u] : coff[u] + cap, :].rearrange(
                        "(g p) e -> p g e", p=P),
                    o[:].rearrange("p (g e) -> p g e", e=NEMB),
                )
    nc.compile()
    return nc


def _ensure_profile_hook():
    try:
        import antenv.axon_hooks  # noqa: F401
        return
    except ImportError:
        pass
    import contextlib, ctypes, sys, types

    so_path = "/opt/axon/libaxon_pjrt.so"
    hook = None
    try:
        lib = ctypes.CDLL(so_path)
        if hasattr(lib, "axon_start_nrt_profile"):
            lib.axon_start_nrt_profile.argtypes = [
                ctypes.POINTER(ctypes.c_int64), ctypes.c_size_t]
            lib.axon_start_nrt_profile.restype = ctypes.c_int64
            lib.axon_stop_nrt_profile.argtypes = [ctypes.c_char_p]
            lib.axon_stop_nrt_profile.restype = ctypes.c_int64

            @contextlib.contextmanager
            def hook(output_dir, device_ids):
                import jax
                jax.devices()
                if device_ids:
                    ids = (ctypes.c_int64 * len(device_ids))(*device_ids)
                    rc = lib.axon_start_nrt_profile(ids, len(device_ids))
                else:
                    rc = lib.axon_start_nrt_profile(None, 0)
                if rc != 0:
                    raise RuntimeError(f"axon_start_nrt_profile rc={rc}")
                try:
                    yield
                finally:
                    lib.axon_stop_nrt_profile(str(output_dir).encode())
    except OSError:
        pass
    mod = types.ModuleType("antenv.axon_hooks")
    mod.get_axon_ntff_profile_hook = lambda: hook
    mod.set_axon_ntff_profile_hook = lambda h: None
    sys.modules["antenv.axon_hooks"] = mod


def _unshard(ntok, caps, cap0, lists, results):
    coff = {0: 0}
    off = cap0
    for u in UNITS:
        coff[u] = off
        off += caps[u]
    acc = np.zeros((ntok, NEMB), np.float32)
    for i in range(N_CORES):
        oc = np.asarray(results[i]["outc"])
        _, pos0 = lists[i][0]
        acc[pos0] = oc[0 : len(pos0)].astype(np.float32)
        for u in UNITS:
            _, pos = lists[i][u]
            acc[pos] = oc[coff[u] : coff[u] + len(pos)].astype(np.float32)
    return acc


def _run(inputs, trace=False):
    _ensure_profile_hook()
    from concourse.bass_utils import run_bass_kernel_spmd

    (per, tab0, tabs, projs, metas16, metas32, caps, cap0, n0, m16off,
     m16w, lists) = _prep_host(inputs)
    shapes = {0: tab0.shape}
    shapes.update({u: tabs[u].shape for u in UNITS})
    nc = _build(per, shapes, projs, caps, cap0, n0, m16off, m16w)

    in_maps = []
    for core in range(N_CORES):
        m = {f"tab{u[0]}_{u[1]}": np.asarray(tabs[u]) for u in UNITS}
        m["tab0"] = np.asarray(tab0)
        m.update({f"proj{i}": np.asarray(projs[i]) for i in (1, 2, 3)})
        m["meta16"] = metas16[core]
        m["meta32"] = metas32[core]
        in_maps.append(m)
    try:
        res = run_bass_kernel_spmd(
            nc, in_maps, core_ids=list(range(N_CORES)), trace=trace
        )
    except Exception:
        import time as _time

        _time.sleep(90)
        res = run_bass_kernel_spmd(
            nc, in_maps, core_ids=list(range(N_CORES)), trace=trace
        )
    x = np.asarray(inputs["x"])
    acc = _unshard(x.size, caps, cap0, lists, res.results)
    full = acc.reshape(*x.shape, NEMB)
    return full, res


def kernel(**inputs) -> np.ndarray:
    out, _ = _run(inputs, trace=False)
    return out
